# revision 1
# baseline (speedup 1.0000x reference)
"""Morphological dilation (depthwise 3x3, additive SE) on 8 TRN2 NeuronCores.

out[b,c,h,w] = max_{dy,dx in {-1,0,1}} ( x[b,c,h+dy,w+dx] + k[c, (dy+1)*3+(dx+1)] )
with zero padding outside the image.

Sharding: batch -> 8 cores (1 image each). Per core, partitions = (h_half, c)
(2*64 = 128), free dim = rows x cols, processed in row chunks.

Math per chunk: the 9 terms z_i = shift_i(x) + k_i are produced, then reduced
with 8 pairwise maxes (VectorE tensor_tensor, fp16 2x mode, all APs
4-byte-aligned). Term production is split three ways to balance engines:
  - term 0 (dy=-1,dx=-1) is precomputed on the host (x2 = xpad + k0) and
    DMA'd in, costing no compute;
  - VectorE tensor_scalar (4x mode) produces the aligned-column terms
    (dx=-1 at col 0, dx=+1 at col 2), 2-3 per chunk;
  - ScalarE ACTIVATE (1x, alignment-free) produces the rest, including the
    odd-column-offset dx=0 terms, 5-6 per chunk.
All shifts are folded into the term-production reads of a zero-padded input
tile xe [rows+2, 226] (fp16); the max chain itself is always offset-0.
"""

import numpy as np

_CACHE = {}

C = 64
H = 224
W = 224
HALF = 112       # rows per h-half
CHUNKS = (12, 28, 28, 28, 16)  # small first chunk = fast ramp; smaller last = short tail
PRE_TERM = 4                   # center term precomputed on host into x2
# On-chip adds per chunk: VectorE gets aligned terms, ScalarE the rest.
# Alternate 2/3 VectorE adds to land at the fractional balance point.
DVE_ADDS_BY_CHUNK = ((1, 7), (1, 7, 3), (1, 7), (1, 7, 3), (1, 7))
ALL_TERMS = (1, 7, 0, 3, 6, 2, 5, 8)  # on-chip terms (everything but PRE_TERM)


def _build():
    import concourse.tile as tile
    import concourse.mybir as mybir
    from concourse import bacc

    f16 = mybir.dt.float16
    f32 = mybir.dt.float32

    nc = bacc.Bacc("TRN2", target_bir_lowering=False, debug=False)
    x_t = nc.dram_tensor("x", [C, H + 2, W + 2], f16, kind="ExternalInput")
    x2_t = nc.dram_tensor("x2", [C, H + 2, W], f16, kind="ExternalInput")
    k_t = nc.dram_tensor("k", [128, 11], f32, kind="ExternalInput")
    o_t = nc.dram_tensor("out", [C, H, W], f16, kind="ExternalOutput")

    RMAX = max(CHUNKS)
    with tile.TileContext(nc) as tc:
        with (
            tc.tile_pool(name="const", bufs=1) as cpool,
            tc.tile_pool(name="xin", bufs=3) as xpool,
            tc.tile_pool(name="x2in", bufs=2) as x2pool,
            tc.tile_pool(name="z", bufs=8) as zpool,
            tc.tile_pool(name="o", bufs=2) as opool,
        ):
            kb = cpool.tile([128, 11], f32)
            nc.gpsimd.dma_start(kb[:], k_t[:])

            starts = [sum(CHUNKS[:i]) for i in range(len(CHUNKS))]

            def load_chunk(ci):
                R, r0 = CHUNKS[ci], starts[ci]
                xe = xpool.tile([128, RMAX + 2, W + 2], f16, tag="xe")
                x2 = x2pool.tile([128, RMAX + 2, W], f16, tag="x2")
                for half in range(2):
                    rows = slice(half * HALF + r0, half * HALF + r0 + R + 2)
                    ps = slice(half * C, half * C + C)
                    nc.sync.dma_start(x2[ps, 0 : R + 2, :], x2_t[:, rows, :])
                    nc.sync.dma_start(xe[ps, 0 : R + 2, :], x_t[:, rows, :])
                return xe, x2

            def add(ci, xe, x2, i, engine):
                R = CHUNKS[ci]
                dyp = i // 3  # row offset inside the haloed tile
                col = i % 3   # column offset in padded coords
                if i in (1, 7):
                    # dx=0 terms come from x2 (= xpad + k4) with delta
                    # constants k_i - k4 (kb cols 9/10) -- aligned reads.
                    src = x2[:, dyp : dyp + R, 0:W]
                    kap = kb[:, 9 + (i == 7) : 10 + (i == 7)]
                else:
                    src = xe[:, dyp : dyp + R, col : col + W]
                    kap = kb[:, i : i + 1]
                z = zpool.tile([128, RMAX, W], f16, tag="z")
                if engine == "v":
                    nc.vector.tensor_scalar_add(z[:, 0:R, :], src, kap)
                else:
                    nc.scalar.add(z[:, 0:R, :], src, kap)
                return z

            # Software-pipelined emission: during chunk ci's max chain,
            # interleave chunk ci+1's VectorE adds so the in-order VectorE
            # stream has fill work if ACT's z production lags the chain.
            xe, x2 = load_chunk(0)
            dve_z = {i: add(0, xe, x2, i, "v") for i in DVE_ADDS_BY_CHUNK[0]}
            for ci, R in enumerate(CHUNKS):
                r0 = starts[ci]
                dve_terms = DVE_ADDS_BY_CHUNK[ci]
                for i in ALL_TERMS:
                    if i not in dve_terms:
                        dve_z[i] = add(ci, xe, x2, i, "s")
                zs, dve_z = dve_z, {}
                nxt = ci + 1
                if nxt < len(CHUNKS):
                    xe_n, x2_n = load_chunk(nxt)
                    nxt_dve = DVE_ADDS_BY_CHUNK[nxt]

                # Max chain (all aligned, 2x). Starts from the precomputed
                # term (ready at DMA time), then consumes VectorE's own z's,
                # then ScalarE's in production order. After chain ops 2/4/6,
                # emit one next-chunk VectorE add as stream fill.
                order = list(dve_terms) + [i for i in ALL_TERMS if i not in dve_terms]
                o = opool.tile([128, RMAX, W], f16, tag="o")
                nc.vector.tensor_max(
                    o[:, 0:R, :], x2[:, 1 : R + 1, :], zs[order[0]][:, 0:R, :]
                )
                for pos, i in enumerate(order[1:], 1):
                    nc.vector.tensor_max(o[:, 0:R, :], o[:, 0:R, :], zs[i][:, 0:R, :])
                    if nxt < len(CHUNKS) and pos in (2, 4, 6):
                        j = (2, 4, 6).index(pos)
                        if j < len(nxt_dve):
                            dve_z[nxt_dve[j]] = add(nxt, xe_n, x2_n, nxt_dve[j], "v")

                for half in range(2):
                    rows = slice(half * HALF + r0, half * HALF + r0 + R)
                    ps = slice(half * C, half * C + C)
                    # Mid-chunk output DMAs issue from the (idle) GpSimd queue
                    # so they never delay input DMAs on the Sync queue; the
                    # last chunk uses the lower-latency HWDGE (sync) queue.
                    eng = nc.sync if nxt == len(CHUNKS) else nc.gpsimd
                    eng.dma_start(o_t[:, rows, :], o[ps, 0:R, :])
                if nxt < len(CHUNKS):
                    xe, x2 = xe_n, x2_n
    nc.finalize()
    return nc


LAST_RESULT = None


def kernel(x, kernel):
    """x: [8,64,224,224] f32; kernel: [1,64,9,1,1] f32 -> [8,64,224,224] f32."""
    global LAST_RESULT
    from concourse.bass_utils import run_bass_kernel_spmd

    if "nc" not in _CACHE:
        _CACHE["nc"] = _build()
    nc = _CACHE["nc"]

    B = x.shape[0]
    xp = np.zeros((B, C, H + 2, W + 2), np.float16)
    xp[:, :, 1 : H + 1, 1 : W + 1] = x
    kb = np.ascontiguousarray(np.asarray(kernel, np.float32).reshape(C, 9))
    kb = np.concatenate([kb, kb], axis=0)  # [128, 9]; partition p = half*64 + c
    # cols 9/10: delta constants k1-k4 and k7-k4 for the x2-based dx=0 terms
    kb = np.concatenate(
        [kb, (kb[:, 1] - kb[:, 4])[:, None], (kb[:, 7] - kb[:, 4])[:, None]], axis=1
    )

    # Precomputed term PRE_TERM: x2[c,r,w] = xpad[c, r, w+colofs] + k[c, PRE_TERM]
    # (fp16 add done in fp32 then rounded, matching on-chip ACT/DVE behavior).
    colofs = PRE_TERM % 3
    xp2 = np.float16(
        np.float32(xp[:, :, :, colofs : colofs + W])
        + np.float32(kb[None, :C, PRE_TERM, None, None])
    )

    in_maps = [{"x": xp[b], "x2": xp2[b], "k": kb} for b in range(B)]
    res = run_bass_kernel_spmd(nc, in_maps, core_ids=list(range(B)))
    LAST_RESULT = res
    out = np.stack([r["out"] for r in res.results], axis=0)
    return out.astype(np.float32)



# revision 2
# speedup vs baseline: 1.1085x; 1.1085x over previous
"""Morphological dilation (depthwise 3x3, additive SE) on 8 TRN2 NeuronCores.

out[b,c,h,w] = max_{dy,dx in {-1,0,1}} ( x[b,c,h+dy,w+dx] + k[c, (dy+1)*3+(dx+1)] )
with zero padding outside the image.

Sharding: batch -> 8 cores (1 image each). Per core, partitions = (h_half, c)
(2*64 = 128), free dim = rows x cols, processed in row chunks.

The entire 9-term max reduction runs as EIGHT fused custom-DVE ops
(ADD_MAX_ANT: out = max(in0 + s0, in1), hand-written 2x_1p uop program, so it
matches tensor_tensor's 2-elem/cycle fp16 throughput while folding the
per-channel kernel constant in for free):

  - xe     = zero-padded input, fp16, [128, 114, 226] (halo rows + cols); the
             six dx=+-1 terms read it at 4B-aligned column offsets 0 / 2.
  - x2     = host-precomputed xpad(col +1) + k4, fp16, [128, 114, 224]; its
             row-0 view seeds the chain (term T4 free), and the two remaining
             dx=0 terms derive from it with delta constants k1-k4 / k7-k4
             (aligned row-shifted reads; a raw odd-column read of xe would
             drop the op to 1x mode).
  - chain: o = AM(xe(dy,dx), k_i, [x2 seed | o]) x6, then o = AM(x2(dy), dk, o) x2.

No ScalarE / tensor_scalar / GpSimd compute at all: DVE runs only 2x_1p ops
(never grabbing the shared 2-port pair), so the GpSimd SWDGE output DMAs
never contend. Input loads are single 128-partition dma_starts (all 16 DMA
ports) on the sync HWDGE queue.
"""

import numpy as np

_CACHE = {}

C = 64
H = 224
W = 224
HALF = 112
ROWS = HALF + 2  # per-half rows incl. 1-row halo each side
CHUNKS = (12, 28, 28, 28, 16)

_ADDMAX_NAME = "ADD_MAX_ANT"


def _register_addmax():
    """Register the fused 2x add-max custom DVE op (idempotent)."""
    from concourse import dve_ops
    from concourse.dve_spec import Spec, Src0, Src1, C0, maxx, lower
    from concourse.dve_uop import (
        AluInp,
        AluOp,
        DelayInp,
        DveOpSpec,
        InpSel,
        OutPath,
        OutSel,
        Trigger,
        UopConfig,
    )

    if _ADDMAX_NAME in dve_ops._SUB_OPCODE_FOR_NAME:
        return next(op for op in dve_ops.OPS if op.name == _ADDMAX_NAME)

    def _ref(in0, in1, s0, s1, imm2):
        return np.maximum(
            in0.astype(np.float32) + s0, in1.astype(np.float32)
        ).astype(np.float32)

    spec = Spec(body=maxx(Src0 + C0, Src1), reference=_ref)

    def _build_2x():
        """Mirror of stock tensor_tensor's 2x_1p program (opcode-table slot 9)
        with the single INSTRUCTION_OP stage split into concrete ADD + MAX.

        Input lanes: 0=SRC_0, 1=SRC_1, 2=SRC_0_HI, 3=SRC_1_HI, 4=CONST_0.
        At blk0: lane0 -> PREV_ALU_OUT, lane(k+1) -> PREV_DELAY_k.
        """
        u = UopConfig()
        u.enable_input(InpSel.SRC_0, 0)
        u.enable_input(InpSel.SRC_1, 1)
        u.enable_input(InpSel.SRC_0_HI, 2)
        u.enable_input(InpSel.SRC_1_HI, 3)
        u.enable_input(InpSel.CONST_0, 4)
        u.require_inp0 = 1
        u.require_inp1 = 1
        u.trigger = (Trigger.SRC_TENSOR_DONE, Trigger.NONE, Trigger.NONE)

        dp = u.datapath_config
        # blk0: a0 = SRC_0 + CONST_0 ; carry SRC_1, SRC_0_HI, SRC_1_HI, CONST_0
        dp[0].enable_alu(AluOp.ADD, AluInp.PREV_ALU_OUT, AluInp.PREV_DELAY_3)
        dp[0].pass_through_delay(0, 1, 2, 3)
        # blk1: r0 = max(a0, SRC_1)
        dp[1].enable_alu(AluOp.MAX, AluInp.PREV_ALU_OUT, AluInp.PREV_DELAY_0)
        dp[1].pass_through_delay(1, 2, 3)
        # blk2: a1 = SRC_0_HI + CONST_0 ; d0 <- r0
        dp[2].enable_alu(AluOp.ADD, AluInp.PREV_DELAY_1, AluInp.PREV_DELAY_3)
        dp[2].enable_delay_from_src(DelayInp.PREV_ALU_OUT, 0)
        dp[2].pass_through_delay(2)
        # blk3: r1 = max(a1, SRC_1_HI) ; carry r0
        dp[3].enable_alu(AluOp.MAX, AluInp.PREV_ALU_OUT, AluInp.PREV_DELAY_2)
        dp[3].pass_through_delay(0)
        # blk4: alu <- r0, d0 <- r1 (swap, as stock does)
        dp[4].enable_alu(AluOp.BYPASS, AluInp.PREV_DELAY_0, AluInp.PREV_DELAY_0)
        dp[4].enable_delay_from_src(DelayInp.PREV_ALU_OUT, 0)
        # blk5..7: bypass r0 down the alu pipe, carry r1
        for b in range(5, 8):
            dp[b].pass_through_alu()
            dp[b].pass_through_delay(0)

        u.enable_output(OutSel.ALU_OUT, OutPath.WR0_LO)
        u.enable_output(OutSel.DELAY_0, OutPath.WR0_HI)
        return u

    class _AddMaxOp:
        name = _ADDMAX_NAME
        subdim = False
        perf_en = {}
        uops_sha = {}

        def __init__(self):
            self.spec = spec
            self._cache = {}

        def compile(self, ver):
            if ver in self._cache:
                return self._cache[ver]
            assert ver == "v3", "ADD_MAX_ANT 2x program authored for TRN2/v3"
            s = DveOpSpec(
                name=self.name,
                opcode=dve_ops.get_dve_sub_opcode(self.name),
                uops=lower(self.spec, ver=ver),
                uops_2x=[_build_2x()],
                rd1_en=True,
                perf_max=1,
            )
            s.validate(ver)
            self._cache[ver] = s
            return s

    op = _AddMaxOp()
    dve_ops.OPS.append(op)
    dve_ops._SUB_OPCODE_FOR_NAME[op.name] = (
        dve_ops._CUSTOM_DVE_ROW_BASE + len(dve_ops.OPS) - 1
    )
    dve_ops.CUSTOM_DVE_SPECS[op.name] = spec
    assert dve_ops._SUB_OPCODE_FOR_NAME[op.name] < 0x20
    return op


def _build():
    import concourse.tile as tile
    import concourse.mybir as mybir
    from concourse import bacc

    f16 = mybir.dt.float16
    f32 = mybir.dt.float32

    am_op = _register_addmax()

    nc = bacc.Bacc("TRN2", target_bir_lowering=False, debug=False)
    xe_t = nc.dram_tensor("xe", [128, ROWS, W + 2], f16, kind="ExternalInput")
    x2_t = nc.dram_tensor("x2", [128, ROWS, W], f16, kind="ExternalInput")
    k_t = nc.dram_tensor("k", [128, 11], f32, kind="ExternalInput")
    o_t = nc.dram_tensor("out", [128, HALF, W], f16, kind="ExternalOutput")

    def am(out, in0, k_col, in1):
        bi = nc.vector._custom_dve(
            am_op, out=out, in0=in0, in1=in1, s0=kb[:, k_col : k_col + 1]
        )
        bi.ins.perf_max = 1
        return bi

    RMAX = max(CHUNKS)
    starts = [sum(CHUNKS[:i]) for i in range(len(CHUNKS))]
    with tile.TileContext(nc) as tc:
        with (
            tc.tile_pool(name="const", bufs=1) as cpool,
            tc.tile_pool(name="xin", bufs=2) as xpool,
            tc.tile_pool(name="x2in", bufs=2) as x2pool,
            tc.tile_pool(name="o", bufs=2) as opool,
        ):
            kb = cpool.tile([128, 11], f32)
            nc.gpsimd.dma_start(kb[:], k_t[:])

            def load_chunk(ci):
                R, r0 = CHUNKS[ci], starts[ci]
                xe = xpool.tile([128, RMAX + 2, W + 2], f16, tag="xe")
                x2 = x2pool.tile([128, RMAX + 2, W], f16, tag="x2")
                nc.sync.dma_start(xe[:, 0 : R + 2, :], xe_t[:, r0 : r0 + R + 2, :])
                nc.sync.dma_start(x2[:, 0 : R + 2, :], x2_t[:, r0 : r0 + R + 2, :])
                return xe, x2

            xe, x2 = load_chunk(0)
            for ci, R in enumerate(CHUNKS):
                r0 = starts[ci]
                nxt = ci + 1
                if nxt < len(CHUNKS):
                    xe_n, x2_n = load_chunk(nxt)

                o = opool.tile([128, RMAX, W], f16, tag="o")
                # terms (dy+1, dx+1, k index): xe cols 0/2 + x2 rows; T4 seeds.
                am(o[:, 0:R, :], xe[:, 0:R, 0:W], 0, x2[:, 1 : R + 1, :])
                am(o[:, 0:R, :], xe[:, 0:R, 2 : W + 2], 2, o[:, 0:R, :])
                am(o[:, 0:R, :], xe[:, 1 : R + 1, 0:W], 3, o[:, 0:R, :])
                am(o[:, 0:R, :], xe[:, 1 : R + 1, 2 : W + 2], 5, o[:, 0:R, :])
                am(o[:, 0:R, :], xe[:, 2 : R + 2, 0:W], 6, o[:, 0:R, :])
                am(o[:, 0:R, :], xe[:, 2 : R + 2, 2 : W + 2], 8, o[:, 0:R, :])
                am(o[:, 0:R, :], x2[:, 0:R, :], 9, o[:, 0:R, :])
                am(o[:, 0:R, :], x2[:, 2 : R + 2, :], 10, o[:, 0:R, :])

                # Mid-chunk output DMAs on the (idle) GpSimd SWDGE queue; the
                # last chunk uses the lower-latency sync HWDGE queue.
                eng = nc.sync if nxt == len(CHUNKS) else nc.gpsimd
                eng.dma_start(o_t[:, r0 : r0 + R, :], o[:, 0:R, :])
                if nxt < len(CHUNKS):
                    xe, x2 = xe_n, x2_n
    nc.finalize()
    return nc


LAST_RESULT = None


def kernel(x, kernel):
    """x: [8,64,224,224] f32; kernel: [1,64,9,1,1] f32 -> [8,64,224,224] f32."""
    global LAST_RESULT
    from concourse.bass_utils import run_bass_kernel_spmd

    if "nc" not in _CACHE:
        _CACHE["nc"] = _build()
    nc = _CACHE["nc"]

    B = x.shape[0]
    kf = np.ascontiguousarray(np.asarray(kernel, np.float32).reshape(C, 9))

    xp = np.zeros((B, C, H + 2, W + 2), np.float16)
    xp[:, :, 1 : H + 1, 1 : W + 1] = x
    # xe: [B, 128, 114, 226], partition p = half*64 + c
    xe = np.concatenate(
        [xp[:, :, 0:ROWS, :], xp[:, :, HALF : HALF + ROWS, :]], axis=1
    )
    # x2 = xpad(col +1) + k4 (fp32 add, fp16 round) -> the three dx=0 terms
    x2full = (
        np.float32(xp[:, :, :, 1 : W + 1]) + kf[None, :, 4, None, None]
    ).astype(np.float16)
    x2 = np.concatenate(
        [x2full[:, :, 0:ROWS, :], x2full[:, :, HALF : HALF + ROWS, :]], axis=1
    )
    # kb cols 0..8 = k0..k8; col 9 = k1-k4; col 10 = k7-k4 (x2 deltas)
    kb = np.concatenate(
        [kf, (kf[:, 1] - kf[:, 4])[:, None], (kf[:, 7] - kf[:, 4])[:, None]], axis=1
    )
    kb = np.concatenate([kb, kb], axis=0)  # [128, 11]

    in_maps = [{"xe": xe[b], "x2": x2[b], "k": kb} for b in range(B)]
    res = run_bass_kernel_spmd(nc, in_maps, core_ids=list(range(B)))
    LAST_RESULT = res
    out = np.stack([r["out"] for r in res.results], axis=0)  # [B, 128, 112, 224]
    out = out.reshape(B, 2, C, HALF, W).transpose(0, 2, 1, 3, 4).reshape(B, C, H, W)
    return out.astype(np.float32)
